# revision 14
# baseline (speedup 1.0000x reference)
"""Trainium2 Bass kernel for the NeuralODE layer (dopri5 fixed-step, 8 steps).

v6: fp8 DoubleRow + interleaved half-batches to hide per-stage latency.

- The two 1024-column half-batches are independent integrations; their stages
  are emitted interleaved (A-st0, B-st0, A-st1, ...) so each half's serial
  tail (layer3 -> k drain -> gating axpy -> next layer1) overlaps the other
  half's matmul work. All stage inputs X_j are finalized one full stage-slot
  before consumption.
- Stage inputs x_j = s + sum_m c_jm k_m are built by a late-formation chain
  into one scratch tile Q per half at stage j-2 (15 DVE axpys/step/half, no
  P partial tiles).
- cf = sum h B_m k_m accumulates inside the stage-6 layer-3 PSUM (W3 scaled
  by C*h*B6; k1/k3/k4/k5 via scaled-identity DoubleRow matmuls; B2=0), then
  one stt per PSUM tile updates the state: s += acc/C.
- The per-step +h*b3 state constant is pre-added as +8*h*b3 to the initial
  projection bias (the tiny transient x-perturbation is far below tolerance).
- k drains ride the Activation engine (Copy, scale=1/(WS*KS)); tanh drains
  emit fp8 with per-stage layer-1 biases folding b3's x-path correction.
"""

import numpy as np
import ml_dtypes

import concourse.bacc as bacc
import concourse.tile as tile
import concourse.mybir as mybir
from concourse.bass_utils import run_bass_kernel_spmd

F32 = mybir.dt.float32
F16 = mybir.dt.float16
F8 = mybir.dt.float8e4
AF = mybir.ActivationFunctionType
OP = mybir.AluOpType
DR = mybir.MatmulPerfMode.DoubleRow

N_CORES = 8
B, IN_DIM, HID = 16384, 256, 512
BSH = B // N_CORES
HALF = 1024
NSTEPS = 8
H = 0.1 * 1 / 8
WS = 16.0                   # fp8 weight scale
KS = 16.0                   # k tiles stored as m/KS

_A = (
    (1 / 5,),
    (3 / 40, 9 / 40),
    (44 / 45, -56 / 15, 32 / 9),
    (19372 / 6561, -25360 / 2187, 64448 / 6561, -212 / 729),
    (9017 / 3168, -355 / 33, 46732 / 5247, 49 / 176, -5103 / 18656),
)
_B = (35 / 384, 0.0, 500 / 1113, 125 / 192, -2187 / 6784, 11 / 84)

KB = HID // 128
KBP = IN_DIM // 128
NC = 512
CPH = HALF // NC

C_CF = 240.0 / (KS * H * _B[3])   # cf psum scale anchor


def build_nc(n_steps=NSTEPS):
    nc = bacc.Bacc("TRN2", target_bir_lowering=False, debug=False,
                   num_devices=N_CORES)

    yT = nc.declare_dram_parameter("yT", [HID, BSH], F32, isOutput=False)
    uT = nc.declare_dram_parameter("uT", [2 * IN_DIM, BSH], F16, isOutput=False)
    w1d = nc.declare_dram_parameter("w1", [16 * 128, 128], F8, isOutput=False)
    w2d = nc.declare_dram_parameter("w2", [16 * 128, 128], F8, isOutput=False)
    w3d = nc.declare_dram_parameter("w3", [16 * 128, 128], F8, isOutput=False)
    w3cd = nc.declare_dram_parameter("w3c", [16 * 128, 128], F8, isOutput=False)
    idd = nc.declare_dram_parameter("idw", [4 * 128, 128], F8, isOutput=False)
    wpd = nc.declare_dram_parameter("wp", [2 * IN_DIM, HID], F16, isOutput=False)
    bpd = nc.declare_dram_parameter("bp", [128, 4], F32, isOutput=False)
    b1ed = nc.declare_dram_parameter("b1e", [128, 24], F32, isOutput=False)
    b2d = nc.declare_dram_parameter("b2", [128, 4], F32, isOutput=False)
    outT = nc.declare_dram_parameter("outT", [HID, BSH], F32, isOutput=True)

    with tile.TileContext(nc) as tc:
        with (
            tc.tile_pool(name="wpool", bufs=1) as wp_,
            tc.tile_pool(name="spool", bufs=1) as sp,
            tc.tile_pool(name="pp", bufs=4, space="PSUM") as pp,
        ):
            # ---- resident weights/biases ----
            wpt = wp_.tile([128, 2 * KBP * 512], F16, tag="wp")
            for kb in range(2 * KBP):
                nc.gpsimd.dma_start(wpt[:, kb * 512:(kb + 1) * 512],
                                    wpd[kb * 128:(kb + 1) * 128, :])
            bpt = wp_.tile([128, 4], F32, tag="bp")
            b1et = wp_.tile([128, 24], F32, tag="b1e")
            b2t = wp_.tile([128, 4], F32, tag="b2")
            nc.gpsimd.dma_start(bpt[:], bpd[:])
            w1t = wp_.tile([128, 16, 128], F8, tag="w1")
            w2t = wp_.tile([128, 16, 128], F8, tag="w2")
            w3t = wp_.tile([128, 16, 128], F8, tag="w3")
            w3ct = wp_.tile([128, 16, 128], F8, tag="w3c")
            idp13 = wp_.tile([128, 2, 128], F8, tag="idp13")
            idp45 = wp_.tile([128, 2, 128], F8, tag="idp45")

            def load_weights():
                for j in range(16):
                    nc.gpsimd.dma_start(w1t[:, j, :],
                                        w1d[j * 128:(j + 1) * 128, :])
                nc.sync.dma_start(b1et[:], b1ed[:])
                for j in range(16):
                    nc.sync.dma_start(w2t[:, j, :],
                                      w2d[j * 128:(j + 1) * 128, :])
                nc.sync.dma_start(b2t[:], b2d[:])
                for j in range(16):
                    nc.gpsimd.dma_start(w3t[:, j, :],
                                        w3d[j * 128:(j + 1) * 128, :])
                for j in range(16):
                    nc.sync.dma_start(w3ct[:, j, :],
                                      w3cd[j * 128:(j + 1) * 128, :])
                nc.gpsimd.dma_start(idp13[:, 0, :], idd[0:128, :])
                nc.gpsimd.dma_start(idp13[:, 1, :], idd[128:256, :])
                nc.sync.dma_start(idp45[:, 0, :], idd[256:384, :])
                nc.sync.dma_start(idp45[:, 1, :], idd[384:512, :])

            # ---- per-half persistent state ----
            def half_state(hf):
                return dict(
                    s=sp.tile([128, KB, HALF], F32, tag=f"s{hf}",
                              name=f"s{hf}"),
                    s8=sp.tile([128, KB, HALF], F8, tag=f"s8{hf}",
                               name=f"s8{hf}"),
                    h1=sp.tile([128, KB, HALF], F8, tag=f"h1{hf}",
                               name=f"h1{hf}"),
                    h2=sp.tile([128, KB, HALF], F8, tag=f"h2{hf}",
                               name=f"h2{hf}"),
                    Q=sp.tile([128, KB, HALF], F16, tag=f"Q{hf}",
                              name=f"Q{hf}"),
                    k8=sp.tile([128, 5 * KB, HALF], F8, tag=f"k8{hf}",
                               name=f"k8{hf}"),
                    X={j: sp.tile([128, KB, HALF], F8, tag=f"X{j}{hf}",
                                  name=f"X{j}{hf}")
                       for j in (2, 3, 4, 5, 6)},
                )

            HS = [half_state(0), half_state(1)]
            uTl = sp.tile([128, 2 * KBP, HALF], F16, tag="uTl")

            def kv(m):
                return (m - 1) * KB

            def layer(hs, groups, drain):
                for mb in range(4):
                    acc = pp.tile([128, HALF], F32, tag="psum", name="acc")
                    for c in range(CPH):
                        out = acc[:, c * NC:(c + 1) * NC]
                        ng = len(groups)
                        for gi, (wt, wb, xt, xb) in enumerate(groups):
                            for p in range(2):
                                wi = wb + p * 8 + mb * 2
                                nc.tensor.matmul(
                                    out,
                                    wt[:, wi:wi + 2, :],
                                    xt[:, xb + 2 * p:xb + 2 * p + 2,
                                       c * NC:(c + 1) * NC],
                                    start=(gi == 0 and p == 0),
                                    stop=(gi == ng - 1 and p == 1),
                                    perf_mode=DR)
                    drain(mb, acc)

            def cf_layer(hs, drain):
                h2, k8 = hs["h2"], hs["k8"]
                for mb in range(4):
                    acc = pp.tile([128, HALF], F32, tag="psum", name="acc")
                    for c in range(CPH):
                        out = acc[:, c * NC:(c + 1) * NC]
                        cs = slice(c * NC, (c + 1) * NC)
                        for p in range(2):
                            wi = p * 8 + mb * 2
                            nc.tensor.matmul(
                                out,
                                w3ct[:, wi:wi + 2, :],
                                h2[:, 2 * p:2 * p + 2, cs],
                                start=(p == 0), stop=False,
                                perf_mode=DR)
                        nc.tensor.matmul(
                            out, idp13[:],
                            k8[:, mb:2 * KB + mb + 1:2 * KB, cs],
                            start=False, stop=False, perf_mode=DR)
                        nc.tensor.matmul(
                            out, idp45[:],
                            k8[:, 3 * KB + mb:4 * KB + mb + 1:KB, cs],
                            start=False, stop=True, perf_mode=DR)
                    drain(mb, acc)

            def emit_proj(hs, c0):
                s, s8 = hs["s"], hs["s8"]
                pairs = [(0, 0), (1, 1), (2, 0), (3, 1)]
                for mb in range(4):
                    acc = pp.tile([128, HALF], F32, tag="psum", name="acc")
                    for c in range(CPH):
                        out = acc[:, c * NC:(c + 1) * NC]
                        for pi, (ub, wb) in enumerate(pairs):
                            nc.tensor.matmul(
                                out,
                                wpt[:, wb * 512 + mb * 128:
                                    wb * 512 + (mb + 1) * 128],
                                uTl[:, ub, c * NC:(c + 1) * NC],
                                start=(pi == 0), stop=(pi == len(pairs) - 1))
                    nc.vector.scalar_tensor_tensor(
                        s[:, mb, :], acc[:], bpt[:, mb:mb + 1], s[:, mb, :],
                        op0=OP.add, op1=OP.add)
                    nc.vector.tensor_copy(s8[:, mb, :], s[:, mb, :])

            def emit_l1(hs, st):
                s8, h1, X = hs["s8"], hs["h1"], hs["X"]
                g1 = [(w1t, 0, s8 if st == 0 else X[st + 1], 0)]

                def drain1(mb, acc, st=st):
                    nc.scalar.activation(
                        h1[:, mb, :], acc[:], AF.Tanh,
                        bias=b1et[:, st * 4 + mb:st * 4 + mb + 1],
                        scale=1.0 / WS)

                layer(hs, g1, drain1)

            def emit_l2(hs):
                h1, h2 = hs["h1"], hs["h2"]

                def drain2(mb, acc):
                    nc.scalar.activation(
                        h2[:, mb, :], acc[:], AF.Tanh,
                        bias=b2t[:, mb:mb + 1], scale=1.0 / WS)

                layer(hs, [(w2t, 0, h1, 0)], drain2)

            def emit_l3(hs, st, last, c0):
                s, s8, h2 = hs["s"], hs["s8"], hs["h2"]
                Q, k8, X = hs["Q"], hs["k8"], hs["X"]

                if st < 5:
                    def drain3(mb, acc, st=st):
                        nc.scalar.activation(
                            k8[:, kv(st + 1) + mb:kv(st + 1) + mb + 1, :],
                            acc[:], AF.Copy, bias=0.0, scale=1.0 / (WS * KS))
                    layer(hs, [(w3t, 0, h2, 0)], drain3)

                    # late-formation chain for x_{st+2} (all k_1..k_{st+1}
                    # exist now); X finalized one stage before consumption
                    j = st + 2
                    cc = [float(KS * H * _A[j - 2][m - 1]) for m in range(1, j)]
                    if st == 0:
                        nc.vector.scalar_tensor_tensor(
                            X[2][:], k8[:, kv(1):kv(1) + KB, :], cc[0], s[:],
                            op0=OP.mult, op1=OP.add)
                    else:
                        nc.vector.scalar_tensor_tensor(
                            Q[:], k8[:, kv(1):kv(1) + KB, :], cc[0], s[:],
                            op0=OP.mult, op1=OP.add)
                        for m in range(2, j - 1):
                            nc.vector.scalar_tensor_tensor(
                                Q[:], k8[:, kv(m):kv(m) + KB, :], cc[m - 1],
                                Q[:], op0=OP.mult, op1=OP.add)
                        nc.vector.scalar_tensor_tensor(
                            X[j][:], k8[:, kv(j - 1):kv(j - 1) + KB, :],
                            cc[j - 2], Q[:], op0=OP.mult, op1=OP.add)
                else:
                    def drain3(mb, acc, last=last, c0=c0):
                        # s8 first and from its own fp8 chain: unblocks the
                        # next step's layer-1 without waiting on the fp32
                        # update (the fp8 view's random-walk drift is far
                        # below tolerance; the fp32 chain stays exact)
                        if not last:
                            nc.vector.scalar_tensor_tensor(
                                s8[:, mb:mb + 1, :], acc[:], 1.0 / C_CF,
                                s8[:, mb:mb + 1, :], op0=OP.mult, op1=OP.add)
                        nc.vector.scalar_tensor_tensor(
                            s[:, mb:mb + 1, :], acc[:], 1.0 / C_CF,
                            s[:, mb:mb + 1, :], op0=OP.mult, op1=OP.add)
                        if last:
                            nc.sync.dma_start(
                                outT[mb * 128:(mb + 1) * 128, c0:c0 + HALF],
                                s[:, mb:mb + 1, :])
                    cf_layer(hs, drain3)

            # ---- emission: proj for both halves, then interleaved stages ----
            for kb in range(2 * KBP):
                e = nc.gpsimd if kb % 2 == 0 else nc.sync
                e.dma_start(uTl[:, kb, :], uT[kb * 128:(kb + 1) * 128, 0:HALF])
            for kb in range(KB):
                e = nc.gpsimd if kb % 2 == 0 else nc.sync
                e.dma_start(HS[0]["s"][:, kb, :],
                            yT[kb * 128:(kb + 1) * 128, 0:HALF])
            load_weights()
            emit_proj(HS[0], 0)
            for kb in range(2 * KBP):
                e = nc.gpsimd if kb % 2 == 0 else nc.sync
                e.dma_start(uTl[:, kb, :],
                            uT[kb * 128:(kb + 1) * 128, HALF:2 * HALF])
            for kb in range(KB):
                e = nc.gpsimd if kb % 2 == 0 else nc.sync
                e.dma_start(HS[1]["s"][:, kb, :],
                            yT[kb * 128:(kb + 1) * 128, HALF:2 * HALF])
            emit_proj(HS[1], HALF)

            # stagger half B by 3 stage-slots so the two halves' expensive
            # step boundaries (state-update chains) alternate
            seq = [(step, st) for step in range(n_steps) for st in range(6)]
            OFF = 3
            for i in range(len(seq) + OFF):
                pair = [(0, i), (1, i - OFF)]
                valid = [(hf, ix) for hf, ix in pair if 0 <= ix < len(seq)]
                for hf, ix in valid:
                    emit_l1(HS[hf], seq[ix][1])
                for hf, ix in valid:
                    emit_l2(HS[hf])
                for hf, ix in valid:
                    step, st = seq[ix]
                    emit_l3(HS[hf], st, step == n_steps - 1, hf * HALF)

    nc.compile()
    return nc


_NC_CACHE = {}


def _get_nc(n_steps=NSTEPS):
    if n_steps not in _NC_CACHE:
        _NC_CACHE[n_steps] = build_nc(n_steps)
    return _NC_CACHE[n_steps]


def _make_in_maps(inputs):
    y = np.asarray(inputs["y"], np.float32)
    u_t = np.asarray(inputs["u_t"], np.float32)
    yT = np.ascontiguousarray(y.T)
    uT = np.ascontiguousarray(u_t.T)
    wp32 = np.asarray(inputs["Wp"], np.float32)
    wp_hi = wp32.astype(np.float16)
    wp_lo = (wp32 - wp_hi.astype(np.float32)).astype(np.float16)
    uT_hi = uT.astype(np.float16)
    uT_lo = (uT - uT_hi.astype(np.float32)).astype(np.float16)
    uT = np.concatenate([uT_hi, uT_lo], axis=0)

    def q8(w):
        return np.asarray(w, np.float64).astype(ml_dtypes.float8_e4m3)

    W1 = np.asarray(inputs["W1"], np.float64)
    W3 = np.asarray(inputs["W3"], np.float64)
    w1q = q8(WS * W1)
    w2q = q8(WS * np.asarray(inputs["W2"], np.float64))
    w3q = q8(WS * W3)
    w3c = q8(C_CF * H * _B[5] * W3)

    def dr_layout(wq):
        # [k, pair*8 + mb*2 + i, m] -> [2048, 128]; lhsT pair slices contiguous
        arr = np.zeros((16, 128, 128), wq.dtype)
        for p in range(2):
            for mb in range(4):
                for i in range(2):
                    arr[p * 8 + mb * 2 + i] = wq[(2 * p + i) * 128:
                                                 (2 * p + i + 1) * 128,
                                                 mb * 128:(mb + 1) * 128]
        return arr.reshape(16 * 128, 128)
    eye = np.eye(128, dtype=np.float64)
    idw = np.concatenate(
        [q8(C_CF * KS * H * _B[m - 1] * eye) for m in (1, 3, 4, 5)], axis=0)
    b1 = np.asarray(inputs["b1"], np.float64)
    b2 = np.asarray(inputs["b2"], np.float64)
    b3 = np.asarray(inputs["b3"], np.float64)
    b3w1 = b3 @ (w1q.astype(np.float64) / WS)
    b1e = np.zeros((6, HID), np.float64)
    b1e[0] = b1
    for st in range(1, 6):
        b1e[st] = b1 + H * float(sum(_A[st - 1])) * b3w1
    # per-step +h*b3 state constant pre-added to the projection bias
    bp_eff = (np.asarray(inputs["bp"], np.float64)
              + NSTEPS * H * b3).astype(np.float32)
    shared = {
        "w1": np.ascontiguousarray(dr_layout(w1q)),
        "w2": np.ascontiguousarray(dr_layout(w2q)),
        "w3": np.ascontiguousarray(dr_layout(w3q)),
        "w3c": np.ascontiguousarray(dr_layout(w3c)),
        "idw": np.ascontiguousarray(idw),
        "wp": np.ascontiguousarray(np.concatenate([wp_hi, wp_lo], axis=0)),
        "bp": np.ascontiguousarray(bp_eff.reshape(4, 128).T),
        "b1e": np.ascontiguousarray(
            b1e.reshape(6, 4, 128).transpose(2, 0, 1).reshape(128, 24)
            .astype(np.float32)),
        "b2": np.ascontiguousarray(b2.astype(np.float32).reshape(4, 128).T),
    }
    in_maps = []
    for i in range(N_CORES):
        sl = slice(i * BSH, (i + 1) * BSH)
        m = dict(shared)
        m["yT"] = np.ascontiguousarray(yT[:, sl])
        m["uT"] = np.ascontiguousarray(uT[:, sl])
        in_maps.append(m)
    return in_maps


def _run(inputs, trace=False, n_steps=NSTEPS):
    nc = _get_nc(n_steps)
    in_maps = _make_in_maps(inputs)
    res = run_bass_kernel_spmd(nc, in_maps, list(range(N_CORES)), trace=trace)
    out = np.empty((HID, B), np.float32)
    for i in range(N_CORES):
        out[:, i * BSH:(i + 1) * BSH] = res.results[i]["outT"]
    return np.ascontiguousarray(out.T), res


def kernel(**inputs) -> np.ndarray:
    out, _ = _run(inputs, trace=False)
    return out


# revision 15
# speedup vs baseline: 1.0101x; 1.0101x over previous
"""Trainium2 Bass kernel for the NeuralODE layer (dopri5 fixed-step, 8 steps).

v6: fp8 DoubleRow + interleaved half-batches to hide per-stage latency.

- The two 1024-column half-batches are independent integrations; their stages
  are emitted interleaved (A-st0, B-st0, A-st1, ...) so each half's serial
  tail (layer3 -> k drain -> gating axpy -> next layer1) overlaps the other
  half's matmul work. All stage inputs X_j are finalized one full stage-slot
  before consumption.
- Stage inputs x_j = s + sum_m c_jm k_m are built by a late-formation chain
  into one scratch tile Q per half at stage j-2 (15 DVE axpys/step/half, no
  P partial tiles).
- cf = sum h B_m k_m accumulates inside the stage-6 layer-3 PSUM (W3 scaled
  by C*h*B6; k1/k3/k4/k5 via scaled-identity DoubleRow matmuls; B2=0), then
  one stt per PSUM tile updates the state: s += acc/C.
- The per-step +h*b3 state constant is pre-added as +8*h*b3 to the initial
  projection bias (the tiny transient x-perturbation is far below tolerance).
- k drains ride the Activation engine (Copy, scale=1/(WS*KS)); tanh drains
  emit fp8 with per-stage layer-1 biases folding b3's x-path correction.
"""

import numpy as np
import ml_dtypes

import concourse.bacc as bacc
import concourse.tile as tile
import concourse.mybir as mybir
from concourse.bass_utils import run_bass_kernel_spmd

F32 = mybir.dt.float32
F16 = mybir.dt.float16
F8 = mybir.dt.float8e4
AF = mybir.ActivationFunctionType
OP = mybir.AluOpType
DR = mybir.MatmulPerfMode.DoubleRow

N_CORES = 8
B, IN_DIM, HID = 16384, 256, 512
BSH = B // N_CORES
HALF = 1024
NSTEPS = 8
H = 0.1 * 1 / 8
WS = 16.0                   # fp8 weight scale
KS = 16.0                   # k tiles stored as m/KS

_A = (
    (1 / 5,),
    (3 / 40, 9 / 40),
    (44 / 45, -56 / 15, 32 / 9),
    (19372 / 6561, -25360 / 2187, 64448 / 6561, -212 / 729),
    (9017 / 3168, -355 / 33, 46732 / 5247, 49 / 176, -5103 / 18656),
)
_B = (35 / 384, 0.0, 500 / 1113, 125 / 192, -2187 / 6784, 11 / 84)

KB = HID // 128
KBP = IN_DIM // 128
NC = 512
CPH = HALF // NC

C_CF = 240.0 / (KS * H * _B[3])   # cf psum scale anchor


def build_nc(n_steps=NSTEPS):
    nc = bacc.Bacc("TRN2", target_bir_lowering=False, debug=False,
                   num_devices=N_CORES)

    yT = nc.declare_dram_parameter("yT", [HID, BSH], F32, isOutput=False)
    uT = nc.declare_dram_parameter("uT", [2 * IN_DIM, BSH], F16, isOutput=False)
    w1d = nc.declare_dram_parameter("w1", [16 * 128, 128], F8, isOutput=False)
    w2d = nc.declare_dram_parameter("w2", [16 * 128, 128], F8, isOutput=False)
    w3d = nc.declare_dram_parameter("w3", [16 * 128, 128], F8, isOutput=False)
    w3cd = nc.declare_dram_parameter("w3c", [16 * 128, 128], F8, isOutput=False)
    idd = nc.declare_dram_parameter("idw", [4 * 128, 128], F8, isOutput=False)
    wpd = nc.declare_dram_parameter("wp", [2 * IN_DIM, HID], F16, isOutput=False)
    bpd = nc.declare_dram_parameter("bp", [128, 4], F32, isOutput=False)
    b1ed = nc.declare_dram_parameter("b1e", [128, 24], F32, isOutput=False)
    b2d = nc.declare_dram_parameter("b2", [128, 4], F32, isOutput=False)
    outT = nc.declare_dram_parameter("outT", [HID, BSH], F32, isOutput=True)

    with tile.TileContext(nc) as tc:
        with (
            tc.tile_pool(name="wpool", bufs=1) as wp_,
            tc.tile_pool(name="spool", bufs=1) as sp,
            tc.tile_pool(name="pp", bufs=4, space="PSUM") as pp,
        ):
            # ---- resident weights/biases ----
            wpt = wp_.tile([128, 2 * KBP * 512], F16, tag="wp")
            for kb in range(2 * KBP):
                nc.gpsimd.dma_start(wpt[:, kb * 512:(kb + 1) * 512],
                                    wpd[kb * 128:(kb + 1) * 128, :])
            bpt = wp_.tile([128, 4], F32, tag="bp")
            b1et = wp_.tile([128, 24], F32, tag="b1e")
            b2t = wp_.tile([128, 4], F32, tag="b2")
            nc.gpsimd.dma_start(bpt[:], bpd[:])
            w1t = wp_.tile([128, 16, 128], F8, tag="w1")
            w2t = wp_.tile([128, 16, 128], F8, tag="w2")
            w3t = wp_.tile([128, 16, 128], F8, tag="w3")
            w3ct = wp_.tile([128, 16, 128], F8, tag="w3c")
            idp13 = wp_.tile([128, 2, 128], F8, tag="idp13")
            idp45 = wp_.tile([128, 2, 128], F8, tag="idp45")

            def load_weights():
                for j in range(16):
                    nc.gpsimd.dma_start(w1t[:, j, :],
                                        w1d[j * 128:(j + 1) * 128, :])
                nc.sync.dma_start(b1et[:], b1ed[:])
                for j in range(16):
                    nc.sync.dma_start(w2t[:, j, :],
                                      w2d[j * 128:(j + 1) * 128, :])
                nc.sync.dma_start(b2t[:], b2d[:])
                for j in range(16):
                    nc.gpsimd.dma_start(w3t[:, j, :],
                                        w3d[j * 128:(j + 1) * 128, :])
                for j in range(16):
                    nc.sync.dma_start(w3ct[:, j, :],
                                      w3cd[j * 128:(j + 1) * 128, :])
                nc.gpsimd.dma_start(idp13[:, 0, :], idd[0:128, :])
                nc.gpsimd.dma_start(idp13[:, 1, :], idd[128:256, :])
                nc.sync.dma_start(idp45[:, 0, :], idd[256:384, :])
                nc.sync.dma_start(idp45[:, 1, :], idd[384:512, :])

            # ---- per-half persistent state ----
            def half_state(hf):
                return dict(
                    s=sp.tile([128, KB, HALF], F32, tag=f"s{hf}",
                              name=f"s{hf}"),
                    s8=sp.tile([128, KB, HALF], F8, tag=f"s8{hf}",
                               name=f"s8{hf}"),
                    h1=sp.tile([128, KB, HALF], F8, tag=f"h1{hf}",
                               name=f"h1{hf}"),
                    h2=sp.tile([128, KB, HALF], F8, tag=f"h2{hf}",
                               name=f"h2{hf}"),
                    Q=sp.tile([128, KB, HALF], F16, tag=f"Q{hf}",
                              name=f"Q{hf}"),
                    k8=sp.tile([128, 5 * KB, HALF], F8, tag=f"k8{hf}",
                               name=f"k8{hf}"),
                    X={j: sp.tile([128, KB, HALF], F8, tag=f"X{j}{hf}",
                                  name=f"X{j}{hf}")
                       for j in (2, 3, 4, 5, 6)},
                )

            HS = [half_state(0), half_state(1)]
            uTl = sp.tile([128, 2 * KBP, HALF], F16, tag="uTl")

            def kv(m):
                return (m - 1) * KB

            def layer(hs, groups, drain):
                for mb in range(4):
                    acc = pp.tile([128, HALF], F32, tag="psum", name="acc")
                    for c in range(CPH):
                        out = acc[:, c * NC:(c + 1) * NC]
                        ng = len(groups)
                        for gi, (wt, wb, xt, xb) in enumerate(groups):
                            for p in range(2):
                                wi = wb + p * 8 + mb * 2
                                nc.tensor.matmul(
                                    out,
                                    wt[:, wi:wi + 2, :],
                                    xt[:, xb + 2 * p:xb + 2 * p + 2,
                                       c * NC:(c + 1) * NC],
                                    start=(gi == 0 and p == 0),
                                    stop=(gi == ng - 1 and p == 1),
                                    perf_mode=DR)
                    drain(mb, acc)

            def cf_layer(hs, drain):
                h2, k8 = hs["h2"], hs["k8"]
                for mb in range(4):
                    acc = pp.tile([128, HALF], F32, tag="psum", name="acc")
                    for c in range(CPH):
                        out = acc[:, c * NC:(c + 1) * NC]
                        cs = slice(c * NC, (c + 1) * NC)
                        for p in range(2):
                            wi = p * 8 + mb * 2
                            nc.tensor.matmul(
                                out,
                                w3ct[:, wi:wi + 2, :],
                                h2[:, 2 * p:2 * p + 2, cs],
                                start=(p == 0), stop=False,
                                perf_mode=DR)
                        nc.tensor.matmul(
                            out, idp13[:],
                            k8[:, mb:2 * KB + mb + 1:2 * KB, cs],
                            start=False, stop=False, perf_mode=DR)
                        nc.tensor.matmul(
                            out, idp45[:],
                            k8[:, 3 * KB + mb:4 * KB + mb + 1:KB, cs],
                            start=False, stop=True, perf_mode=DR)
                    drain(mb, acc)

            def emit_proj(hs, c0):
                s, s8 = hs["s"], hs["s8"]
                pairs = [(0, 0), (1, 1), (2, 0), (3, 1), (0, 2), (1, 3)]
                for mb in range(4):
                    acc = pp.tile([128, HALF], F32, tag="psum", name="acc")
                    for c in range(CPH):
                        out = acc[:, c * NC:(c + 1) * NC]
                        for pi, (ub, wb) in enumerate(pairs):
                            nc.tensor.matmul(
                                out,
                                wpt[:, wb * 512 + mb * 128:
                                    wb * 512 + (mb + 1) * 128],
                                uTl[:, ub, c * NC:(c + 1) * NC],
                                start=(pi == 0), stop=(pi == len(pairs) - 1))
                    nc.vector.scalar_tensor_tensor(
                        s[:, mb, :], acc[:], bpt[:, mb:mb + 1], s[:, mb, :],
                        op0=OP.add, op1=OP.add)
                    nc.vector.tensor_copy(s8[:, mb, :], s[:, mb, :])

            def emit_l1(hs, st):
                s8, h1, X = hs["s8"], hs["h1"], hs["X"]
                g1 = [(w1t, 0, s8 if st == 0 else X[st + 1], 0)]

                def drain1(mb, acc, st=st):
                    nc.scalar.activation(
                        h1[:, mb, :], acc[:], AF.Tanh,
                        bias=b1et[:, st * 4 + mb:st * 4 + mb + 1],
                        scale=1.0 / WS)

                layer(hs, g1, drain1)

            def emit_l2(hs):
                h1, h2 = hs["h1"], hs["h2"]

                def drain2(mb, acc):
                    nc.scalar.activation(
                        h2[:, mb, :], acc[:], AF.Tanh,
                        bias=b2t[:, mb:mb + 1], scale=1.0 / WS)

                layer(hs, [(w2t, 0, h1, 0)], drain2)

            def emit_l3(hs, st, last, c0):
                s, s8, h2 = hs["s"], hs["s8"], hs["h2"]
                Q, k8, X = hs["Q"], hs["k8"], hs["X"]

                if st < 5:
                    def drain3(mb, acc, st=st):
                        nc.scalar.activation(
                            k8[:, kv(st + 1) + mb:kv(st + 1) + mb + 1, :],
                            acc[:], AF.Copy, bias=0.0, scale=1.0 / (WS * KS))
                    layer(hs, [(w3t, 0, h2, 0)], drain3)

                    # late-formation chain for x_{st+2} (all k_1..k_{st+1}
                    # exist now); X finalized one stage before consumption
                    j = st + 2
                    cc = [float(KS * H * _A[j - 2][m - 1]) for m in range(1, j)]
                    if st == 0:
                        nc.vector.scalar_tensor_tensor(
                            X[2][:], k8[:, kv(1):kv(1) + KB, :], cc[0], s[:],
                            op0=OP.mult, op1=OP.add)
                    else:
                        nc.vector.scalar_tensor_tensor(
                            Q[:], k8[:, kv(1):kv(1) + KB, :], cc[0], s[:],
                            op0=OP.mult, op1=OP.add)
                        for m in range(2, j - 1):
                            nc.vector.scalar_tensor_tensor(
                                Q[:], k8[:, kv(m):kv(m) + KB, :], cc[m - 1],
                                Q[:], op0=OP.mult, op1=OP.add)
                        nc.vector.scalar_tensor_tensor(
                            X[j][:], k8[:, kv(j - 1):kv(j - 1) + KB, :],
                            cc[j - 2], Q[:], op0=OP.mult, op1=OP.add)
                else:
                    def drain3(mb, acc, last=last, c0=c0):
                        nc.vector.scalar_tensor_tensor(
                            s[:, mb:mb + 1, :], acc[:], 1.0 / C_CF,
                            s[:, mb:mb + 1, :], op0=OP.mult, op1=OP.add)
                        if not last:
                            nc.vector.tensor_copy(
                                s8[:, mb:mb + 1, :], s[:, mb:mb + 1, :])
                        else:
                            nc.sync.dma_start(
                                outT[mb * 128:(mb + 1) * 128, c0:c0 + HALF],
                                s[:, mb:mb + 1, :])
                    cf_layer(hs, drain3)

            # ---- emission: proj for both halves, then interleaved stages ----
            for kb in range(2 * KBP):
                e = nc.gpsimd if kb % 2 == 0 else nc.sync
                e.dma_start(uTl[:, kb, :], uT[kb * 128:(kb + 1) * 128, 0:HALF])
            for kb in range(KB):
                e = nc.gpsimd if kb % 2 == 0 else nc.sync
                e.dma_start(HS[0]["s"][:, kb, :],
                            yT[kb * 128:(kb + 1) * 128, 0:HALF])
            load_weights()
            emit_proj(HS[0], 0)
            for kb in range(2 * KBP):
                e = nc.gpsimd if kb % 2 == 0 else nc.sync
                e.dma_start(uTl[:, kb, :],
                            uT[kb * 128:(kb + 1) * 128, HALF:2 * HALF])
            for kb in range(KB):
                e = nc.gpsimd if kb % 2 == 0 else nc.sync
                e.dma_start(HS[1]["s"][:, kb, :],
                            yT[kb * 128:(kb + 1) * 128, HALF:2 * HALF])
            emit_proj(HS[1], HALF)

            # stagger half B by 3 stage-slots so the two halves' expensive
            # step boundaries (state-update chains) alternate
            seq = [(step, st) for step in range(n_steps) for st in range(6)]
            OFF = 3
            for i in range(len(seq) + OFF):
                pair = [(0, i), (1, i - OFF)]
                valid = [(hf, ix) for hf, ix in pair if 0 <= ix < len(seq)]
                for hf, ix in valid:
                    emit_l1(HS[hf], seq[ix][1])
                for hf, ix in valid:
                    emit_l2(HS[hf])
                for hf, ix in valid:
                    step, st = seq[ix]
                    emit_l3(HS[hf], st, step == n_steps - 1, hf * HALF)

    nc.compile()
    return nc


_NC_CACHE = {}


def _get_nc(n_steps=NSTEPS):
    if n_steps not in _NC_CACHE:
        _NC_CACHE[n_steps] = build_nc(n_steps)
    return _NC_CACHE[n_steps]


def _make_in_maps(inputs):
    y = np.asarray(inputs["y"], np.float32)
    u_t = np.asarray(inputs["u_t"], np.float32)
    yT = np.ascontiguousarray(y.T)
    uT = np.ascontiguousarray(u_t.T)
    wp32 = np.asarray(inputs["Wp"], np.float32)
    wp_hi = wp32.astype(np.float16)
    wp_lo = (wp32 - wp_hi.astype(np.float32)).astype(np.float16)
    uT_hi = uT.astype(np.float16)
    uT_lo = (uT - uT_hi.astype(np.float32)).astype(np.float16)
    uT = np.concatenate([uT_hi, uT_lo], axis=0)

    def q8(w):
        return np.asarray(w, np.float64).astype(ml_dtypes.float8_e4m3)

    W1 = np.asarray(inputs["W1"], np.float64)
    W3 = np.asarray(inputs["W3"], np.float64)
    w1q = q8(WS * W1)
    w2q = q8(WS * np.asarray(inputs["W2"], np.float64))
    w3q = q8(WS * W3)
    w3c = q8(C_CF * H * _B[5] * W3)

    def dr_layout(wq):
        # [k, pair*8 + mb*2 + i, m] -> [2048, 128]; lhsT pair slices contiguous
        arr = np.zeros((16, 128, 128), wq.dtype)
        for p in range(2):
            for mb in range(4):
                for i in range(2):
                    arr[p * 8 + mb * 2 + i] = wq[(2 * p + i) * 128:
                                                 (2 * p + i + 1) * 128,
                                                 mb * 128:(mb + 1) * 128]
        return arr.reshape(16 * 128, 128)
    eye = np.eye(128, dtype=np.float64)
    idw = np.concatenate(
        [q8(C_CF * KS * H * _B[m - 1] * eye) for m in (1, 3, 4, 5)], axis=0)
    b1 = np.asarray(inputs["b1"], np.float64)
    b2 = np.asarray(inputs["b2"], np.float64)
    b3 = np.asarray(inputs["b3"], np.float64)
    b3w1 = b3 @ (w1q.astype(np.float64) / WS)
    b1e = np.zeros((6, HID), np.float64)
    b1e[0] = b1
    for st in range(1, 6):
        b1e[st] = b1 + H * float(sum(_A[st - 1])) * b3w1
    # per-step +h*b3 state constant pre-added to the projection bias
    bp_eff = (np.asarray(inputs["bp"], np.float64)
              + NSTEPS * H * b3).astype(np.float32)
    shared = {
        "w1": np.ascontiguousarray(dr_layout(w1q)),
        "w2": np.ascontiguousarray(dr_layout(w2q)),
        "w3": np.ascontiguousarray(dr_layout(w3q)),
        "w3c": np.ascontiguousarray(dr_layout(w3c)),
        "idw": np.ascontiguousarray(idw),
        "wp": np.ascontiguousarray(np.concatenate([wp_hi, wp_lo], axis=0)),
        "bp": np.ascontiguousarray(bp_eff.reshape(4, 128).T),
        "b1e": np.ascontiguousarray(
            b1e.reshape(6, 4, 128).transpose(2, 0, 1).reshape(128, 24)
            .astype(np.float32)),
        "b2": np.ascontiguousarray(b2.astype(np.float32).reshape(4, 128).T),
    }
    in_maps = []
    for i in range(N_CORES):
        sl = slice(i * BSH, (i + 1) * BSH)
        m = dict(shared)
        m["yT"] = np.ascontiguousarray(yT[:, sl])
        m["uT"] = np.ascontiguousarray(uT[:, sl])
        in_maps.append(m)
    return in_maps


def _run(inputs, trace=False, n_steps=NSTEPS):
    nc = _get_nc(n_steps)
    in_maps = _make_in_maps(inputs)
    res = run_bass_kernel_spmd(nc, in_maps, list(range(N_CORES)), trace=trace)
    out = np.empty((HID, B), np.float32)
    for i in range(N_CORES):
        out[:, i * BSH:(i + 1) * BSH] = res.results[i]["outT"]
    return np.ascontiguousarray(out.T), res


def kernel(**inputs) -> np.ndarray:
    out, _ = _run(inputs, trace=False)
    return out


# revision 17
# speedup vs baseline: 1.0153x; 1.0051x over previous
"""Trainium2 Bass kernel for the NeuralODE layer (dopri5 fixed-step, 8 steps).

v6: fp8 DoubleRow + interleaved half-batches to hide per-stage latency.

- The two 1024-column half-batches are independent integrations; their stages
  are emitted interleaved (A-st0, B-st0, A-st1, ...) so each half's serial
  tail (layer3 -> k drain -> gating axpy -> next layer1) overlaps the other
  half's matmul work. All stage inputs X_j are finalized one full stage-slot
  before consumption.
- Stage inputs x_j = s + sum_m c_jm k_m are built by a late-formation chain
  into one scratch tile Q per half at stage j-2 (15 DVE axpys/step/half, no
  P partial tiles).
- cf = sum h B_m k_m accumulates inside the stage-6 layer-3 PSUM (W3 scaled
  by C*h*B6; k1/k3/k4/k5 via scaled-identity DoubleRow matmuls; B2=0), then
  one stt per PSUM tile updates the state: s += acc/C.
- The per-step +h*b3 state constant is pre-added as +8*h*b3 to the initial
  projection bias (the tiny transient x-perturbation is far below tolerance).
- k drains ride the Activation engine (Copy, scale=1/(WS*KS)); tanh drains
  emit fp8 with per-stage layer-1 biases folding b3's x-path correction.
"""

import numpy as np
import ml_dtypes

import concourse.bacc as bacc
import concourse.tile as tile
import concourse.mybir as mybir
from concourse.bass_utils import run_bass_kernel_spmd

F32 = mybir.dt.float32
F16 = mybir.dt.float16
F8 = mybir.dt.float8e4
AF = mybir.ActivationFunctionType
OP = mybir.AluOpType
DR = mybir.MatmulPerfMode.DoubleRow

N_CORES = 8
B, IN_DIM, HID = 16384, 256, 512
BSH = B // N_CORES
HALF = 1024
NSTEPS = 8
H = 0.1 * 1 / 8
WS = 16.0                   # fp8 weight scale
KS = 16.0                   # k tiles stored as m/KS

_A = (
    (1 / 5,),
    (3 / 40, 9 / 40),
    (44 / 45, -56 / 15, 32 / 9),
    (19372 / 6561, -25360 / 2187, 64448 / 6561, -212 / 729),
    (9017 / 3168, -355 / 33, 46732 / 5247, 49 / 176, -5103 / 18656),
)
_B = (35 / 384, 0.0, 500 / 1113, 125 / 192, -2187 / 6784, 11 / 84)

KB = HID // 128
KBP = IN_DIM // 128
NC = 512
CPH = HALF // NC

C_CF = 240.0 / (KS * H * _B[3])   # cf psum scale anchor


def build_nc(n_steps=NSTEPS):
    nc = bacc.Bacc("TRN2", target_bir_lowering=False, debug=False,
                   num_devices=N_CORES)

    yT = nc.declare_dram_parameter("yT", [HID, BSH], F32, isOutput=False)
    uT = nc.declare_dram_parameter("uT", [2 * IN_DIM, BSH], F16, isOutput=False)
    w1d = nc.declare_dram_parameter("w1", [16 * 128, 128], F8, isOutput=False)
    w2d = nc.declare_dram_parameter("w2", [16 * 128, 128], F8, isOutput=False)
    w3d = nc.declare_dram_parameter("w3", [16 * 128, 128], F8, isOutput=False)
    w3cd = nc.declare_dram_parameter("w3c", [16 * 128, 128], F8, isOutput=False)
    idd = nc.declare_dram_parameter("idw", [4 * 128, 128], F8, isOutput=False)
    wpd = nc.declare_dram_parameter("wp", [2 * IN_DIM, HID], F16, isOutput=False)
    bpd = nc.declare_dram_parameter("bp", [128, 4], F32, isOutput=False)
    b1ed = nc.declare_dram_parameter("b1e", [128, 24], F32, isOutput=False)
    b2d = nc.declare_dram_parameter("b2", [128, 4], F32, isOutput=False)
    outT = nc.declare_dram_parameter("outT", [HID, BSH], F32, isOutput=True)

    with tile.TileContext(nc) as tc:
        with (
            tc.tile_pool(name="wpool", bufs=1) as wp_,
            tc.tile_pool(name="spool", bufs=1) as sp,
            tc.tile_pool(name="pp", bufs=4, space="PSUM") as pp,
        ):
            # ---- resident weights/biases ----
            wpt = wp_.tile([128, 2 * KBP * 512], F16, tag="wp")
            for kb in range(2 * KBP):
                nc.gpsimd.dma_start(wpt[:, kb * 512:(kb + 1) * 512],
                                    wpd[kb * 128:(kb + 1) * 128, :])
            bpt = wp_.tile([128, 4], F32, tag="bp")
            b1et = wp_.tile([128, 24], F32, tag="b1e")
            b2t = wp_.tile([128, 4], F32, tag="b2")
            nc.gpsimd.dma_start(bpt[:], bpd[:])
            w1t = wp_.tile([128, 16, 128], F8, tag="w1")
            w2t = wp_.tile([128, 16, 128], F8, tag="w2")
            w3t = wp_.tile([128, 16, 128], F8, tag="w3")
            w3ct = wp_.tile([128, 16, 128], F8, tag="w3c")
            idp13 = wp_.tile([128, 2, 128], F8, tag="idp13")
            idp45 = wp_.tile([128, 2, 128], F8, tag="idp45")

            def load_weights():
                for j in range(16):
                    nc.gpsimd.dma_start(w1t[:, j, :],
                                        w1d[j * 128:(j + 1) * 128, :])
                nc.sync.dma_start(b1et[:], b1ed[:])
                for j in range(16):
                    nc.sync.dma_start(w2t[:, j, :],
                                      w2d[j * 128:(j + 1) * 128, :])
                nc.sync.dma_start(b2t[:], b2d[:])
                for j in range(16):
                    nc.gpsimd.dma_start(w3t[:, j, :],
                                        w3d[j * 128:(j + 1) * 128, :])
                for j in range(16):
                    nc.sync.dma_start(w3ct[:, j, :],
                                      w3cd[j * 128:(j + 1) * 128, :])
                nc.gpsimd.dma_start(idp13[:, 0, :], idd[0:128, :])
                nc.gpsimd.dma_start(idp13[:, 1, :], idd[128:256, :])
                nc.sync.dma_start(idp45[:, 0, :], idd[256:384, :])
                nc.sync.dma_start(idp45[:, 1, :], idd[384:512, :])

            # ---- per-half persistent state ----
            def half_state(hf):
                return dict(
                    s=sp.tile([128, KB, HALF], F32, tag=f"s{hf}",
                              name=f"s{hf}"),
                    s8=sp.tile([128, KB, HALF], F8, tag=f"s8{hf}",
                               name=f"s8{hf}"),
                    h1=sp.tile([128, KB, HALF], F8, tag=f"h1{hf}",
                               name=f"h1{hf}"),
                    h2=sp.tile([128, KB, HALF], F8, tag=f"h2{hf}",
                               name=f"h2{hf}"),
                    Qa=sp.tile([128, KB, HALF], F16, tag=f"Qa{hf}",
                               name=f"Qa{hf}"),
                    Qb=sp.tile([128, KB, HALF], F16, tag=f"Qb{hf}",
                               name=f"Qb{hf}"),
                    k8=sp.tile([128, 5 * KB, HALF], F8, tag=f"k8{hf}",
                               name=f"k8{hf}"),
                    X={j: sp.tile([128, KB, HALF], F8, tag=f"X{j}{hf}",
                                  name=f"X{j}{hf}")
                       for j in (2, 3, 4, 5, 6)},
                )

            HS = [half_state(0), half_state(1)]
            uTl = sp.tile([128, 2 * KBP, HALF], F16, tag="uTl")

            def kv(m):
                return (m - 1) * KB

            def layer(hs, groups, drain):
                for mb in range(4):
                    acc = pp.tile([128, HALF], F32, tag="psum", name="acc")
                    for c in range(CPH):
                        out = acc[:, c * NC:(c + 1) * NC]
                        ng = len(groups)
                        for gi, (wt, wb, xt, xb) in enumerate(groups):
                            for p in range(2):
                                wi = wb + p * 8 + mb * 2
                                nc.tensor.matmul(
                                    out,
                                    wt[:, wi:wi + 2, :],
                                    xt[:, xb + 2 * p:xb + 2 * p + 2,
                                       c * NC:(c + 1) * NC],
                                    start=(gi == 0 and p == 0),
                                    stop=(gi == ng - 1 and p == 1),
                                    perf_mode=DR)
                    drain(mb, acc)

            def cf_layer(hs, drain):
                h2, k8 = hs["h2"], hs["k8"]
                for mb in range(4):
                    acc = pp.tile([128, HALF], F32, tag="psum", name="acc")
                    for c in range(CPH):
                        out = acc[:, c * NC:(c + 1) * NC]
                        cs = slice(c * NC, (c + 1) * NC)
                        for p in range(2):
                            wi = p * 8 + mb * 2
                            nc.tensor.matmul(
                                out,
                                w3ct[:, wi:wi + 2, :],
                                h2[:, 2 * p:2 * p + 2, cs],
                                start=(p == 0), stop=False,
                                perf_mode=DR)
                        nc.tensor.matmul(
                            out, idp13[:],
                            k8[:, mb:2 * KB + mb + 1:2 * KB, cs],
                            start=False, stop=False, perf_mode=DR)
                        nc.tensor.matmul(
                            out, idp45[:],
                            k8[:, 3 * KB + mb:4 * KB + mb + 1:KB, cs],
                            start=False, stop=True, perf_mode=DR)
                    drain(mb, acc)

            def emit_proj(hs, c0):
                s, s8 = hs["s"], hs["s8"]
                pairs = [(0, 0), (1, 1), (2, 0), (3, 1), (0, 2), (1, 3)]
                for mb in range(4):
                    acc = pp.tile([128, HALF], F32, tag="psum", name="acc")
                    for c in range(CPH):
                        out = acc[:, c * NC:(c + 1) * NC]
                        for pi, (ub, wb) in enumerate(pairs):
                            nc.tensor.matmul(
                                out,
                                wpt[:, wb * 512 + mb * 128:
                                    wb * 512 + (mb + 1) * 128],
                                uTl[:, ub, c * NC:(c + 1) * NC],
                                start=(pi == 0), stop=(pi == len(pairs) - 1))
                    nc.vector.scalar_tensor_tensor(
                        s[:, mb, :], acc[:], bpt[:, mb:mb + 1], s[:, mb, :],
                        op0=OP.add, op1=OP.add)
                    nc.vector.tensor_copy(s8[:, mb, :], s[:, mb, :])

            def emit_l1(hs, st):
                s8, h1, X = hs["s8"], hs["h1"], hs["X"]
                g1 = [(w1t, 0, s8 if st == 0 else X[st + 1], 0)]

                def drain1(mb, acc, st=st):
                    nc.scalar.activation(
                        h1[:, mb, :], acc[:], AF.Tanh,
                        bias=b1et[:, st * 4 + mb:st * 4 + mb + 1],
                        scale=1.0 / WS)

                layer(hs, g1, drain1)

            def emit_l2(hs):
                h1, h2 = hs["h1"], hs["h2"]

                def drain2(mb, acc):
                    nc.scalar.activation(
                        h2[:, mb, :], acc[:], AF.Tanh,
                        bias=b2t[:, mb:mb + 1], scale=1.0 / WS)

                layer(hs, [(w2t, 0, h1, 0)], drain2)

            def emit_l3(hs, st, last, c0):
                s, s8, h2 = hs["s"], hs["s8"], hs["h2"]
                k8, X = hs["k8"], hs["X"]

                if st < 5:
                    def drain3(mb, acc, st=st):
                        nc.scalar.activation(
                            k8[:, kv(st + 1) + mb:kv(st + 1) + mb + 1, :],
                            acc[:], AF.Copy, bias=0.0, scale=1.0 / (WS * KS))
                    layer(hs, [(w3t, 0, h2, 0)], drain3)

                    # x_j chains spread flat across stages on two scratch
                    # tiles (same op sequence as the bunched form, so math is
                    # bit-identical); the consumer-critical X op goes first
                    Qa, Qb = hs["Qa"], hs["Qb"]

                    def km(m):
                        return k8[:, kv(m):kv(m) + KB, :]

                    def ax(dst, m, j, src_):
                        nc.vector.scalar_tensor_tensor(
                            dst[:], km(m), float(KS * H * _A[j - 2][m - 1]),
                            src_[:], op0=OP.mult, op1=OP.add)

                    if st == 0:
                        ax(X[2], 1, 2, s)
                        ax(Qa, 1, 3, s)
                    elif st == 1:
                        ax(X[3], 2, 3, Qa)
                        ax(Qb, 1, 4, s)
                        ax(Qb, 2, 4, Qb)
                    elif st == 2:
                        ax(X[4], 3, 4, Qb)
                        ax(Qa, 1, 5, s)
                        ax(Qa, 2, 5, Qa)
                        ax(Qa, 3, 5, Qa)
                    elif st == 3:
                        ax(X[5], 4, 5, Qa)
                        ax(Qb, 1, 6, s)
                        ax(Qb, 2, 6, Qb)
                        ax(Qb, 3, 6, Qb)
                    else:
                        ax(Qb, 4, 6, Qb)
                        ax(X[6], 5, 6, Qb)
                else:
                    def drain3(mb, acc, last=last, c0=c0):
                        nc.vector.scalar_tensor_tensor(
                            s[:, mb:mb + 1, :], acc[:], 1.0 / C_CF,
                            s[:, mb:mb + 1, :], op0=OP.mult, op1=OP.add)
                        if not last:
                            nc.vector.tensor_copy(
                                s8[:, mb:mb + 1, :], s[:, mb:mb + 1, :])
                        else:
                            nc.sync.dma_start(
                                outT[mb * 128:(mb + 1) * 128, c0:c0 + HALF],
                                s[:, mb:mb + 1, :])
                    cf_layer(hs, drain3)

            # ---- emission: proj for both halves, then interleaved stages ----
            for kb in range(2 * KBP):
                e = nc.gpsimd if kb % 2 == 0 else nc.sync
                e.dma_start(uTl[:, kb, :], uT[kb * 128:(kb + 1) * 128, 0:HALF])
            for kb in range(KB):
                e = nc.gpsimd if kb % 2 == 0 else nc.sync
                e.dma_start(HS[0]["s"][:, kb, :],
                            yT[kb * 128:(kb + 1) * 128, 0:HALF])
            load_weights()
            emit_proj(HS[0], 0)
            for kb in range(2 * KBP):
                e = nc.gpsimd if kb % 2 == 0 else nc.sync
                e.dma_start(uTl[:, kb, :],
                            uT[kb * 128:(kb + 1) * 128, HALF:2 * HALF])
            for kb in range(KB):
                e = nc.gpsimd if kb % 2 == 0 else nc.sync
                e.dma_start(HS[1]["s"][:, kb, :],
                            yT[kb * 128:(kb + 1) * 128, HALF:2 * HALF])
            emit_proj(HS[1], HALF)

            # stagger half B by 3 stage-slots so the two halves' expensive
            # step boundaries (state-update chains) alternate
            seq = [(step, st) for step in range(n_steps) for st in range(6)]
            OFF = 3
            for i in range(len(seq) + OFF):
                pair = [(0, i), (1, i - OFF)]
                valid = [(hf, ix) for hf, ix in pair if 0 <= ix < len(seq)]
                for hf, ix in valid:
                    emit_l1(HS[hf], seq[ix][1])
                for hf, ix in valid:
                    emit_l2(HS[hf])
                for hf, ix in valid:
                    step, st = seq[ix]
                    emit_l3(HS[hf], st, step == n_steps - 1, hf * HALF)

    nc.compile()
    return nc


_NC_CACHE = {}


def _get_nc(n_steps=NSTEPS):
    if n_steps not in _NC_CACHE:
        _NC_CACHE[n_steps] = build_nc(n_steps)
    return _NC_CACHE[n_steps]


def _make_in_maps(inputs):
    y = np.asarray(inputs["y"], np.float32)
    u_t = np.asarray(inputs["u_t"], np.float32)
    yT = np.ascontiguousarray(y.T)
    uT = np.ascontiguousarray(u_t.T)
    wp32 = np.asarray(inputs["Wp"], np.float32)
    wp_hi = wp32.astype(np.float16)
    wp_lo = (wp32 - wp_hi.astype(np.float32)).astype(np.float16)
    uT_hi = uT.astype(np.float16)
    uT_lo = (uT - uT_hi.astype(np.float32)).astype(np.float16)
    uT = np.concatenate([uT_hi, uT_lo], axis=0)

    def q8(w):
        return np.asarray(w, np.float64).astype(ml_dtypes.float8_e4m3)

    W1 = np.asarray(inputs["W1"], np.float64)
    W3 = np.asarray(inputs["W3"], np.float64)
    w1q = q8(WS * W1)
    w2q = q8(WS * np.asarray(inputs["W2"], np.float64))
    w3q = q8(WS * W3)
    w3c = q8(C_CF * H * _B[5] * W3)

    def dr_layout(wq):
        # [k, pair*8 + mb*2 + i, m] -> [2048, 128]; lhsT pair slices contiguous
        arr = np.zeros((16, 128, 128), wq.dtype)
        for p in range(2):
            for mb in range(4):
                for i in range(2):
                    arr[p * 8 + mb * 2 + i] = wq[(2 * p + i) * 128:
                                                 (2 * p + i + 1) * 128,
                                                 mb * 128:(mb + 1) * 128]
        return arr.reshape(16 * 128, 128)
    eye = np.eye(128, dtype=np.float64)
    idw = np.concatenate(
        [q8(C_CF * KS * H * _B[m - 1] * eye) for m in (1, 3, 4, 5)], axis=0)
    b1 = np.asarray(inputs["b1"], np.float64)
    b2 = np.asarray(inputs["b2"], np.float64)
    b3 = np.asarray(inputs["b3"], np.float64)
    b3w1 = b3 @ (w1q.astype(np.float64) / WS)
    b1e = np.zeros((6, HID), np.float64)
    b1e[0] = b1
    for st in range(1, 6):
        b1e[st] = b1 + H * float(sum(_A[st - 1])) * b3w1
    # per-step +h*b3 state constant pre-added to the projection bias
    bp_eff = (np.asarray(inputs["bp"], np.float64)
              + NSTEPS * H * b3).astype(np.float32)
    shared = {
        "w1": np.ascontiguousarray(dr_layout(w1q)),
        "w2": np.ascontiguousarray(dr_layout(w2q)),
        "w3": np.ascontiguousarray(dr_layout(w3q)),
        "w3c": np.ascontiguousarray(dr_layout(w3c)),
        "idw": np.ascontiguousarray(idw),
        "wp": np.ascontiguousarray(np.concatenate([wp_hi, wp_lo], axis=0)),
        "bp": np.ascontiguousarray(bp_eff.reshape(4, 128).T),
        "b1e": np.ascontiguousarray(
            b1e.reshape(6, 4, 128).transpose(2, 0, 1).reshape(128, 24)
            .astype(np.float32)),
        "b2": np.ascontiguousarray(b2.astype(np.float32).reshape(4, 128).T),
    }
    in_maps = []
    for i in range(N_CORES):
        sl = slice(i * BSH, (i + 1) * BSH)
        m = dict(shared)
        m["yT"] = np.ascontiguousarray(yT[:, sl])
        m["uT"] = np.ascontiguousarray(uT[:, sl])
        in_maps.append(m)
    return in_maps


def _run(inputs, trace=False, n_steps=NSTEPS):
    nc = _get_nc(n_steps)
    in_maps = _make_in_maps(inputs)
    res = run_bass_kernel_spmd(nc, in_maps, list(range(N_CORES)), trace=trace)
    out = np.empty((HID, B), np.float32)
    for i in range(N_CORES):
        out[:, i * BSH:(i + 1) * BSH] = res.results[i]["outT"]
    return np.ascontiguousarray(out.T), res


def kernel(**inputs) -> np.ndarray:
    out, _ = _run(inputs, trace=False)
    return out


# revision 18
# speedup vs baseline: 1.0190x; 1.0037x over previous
"""Trainium2 Bass kernel for the NeuralODE layer (dopri5 fixed-step, 8 steps).

v6: fp8 DoubleRow + interleaved half-batches to hide per-stage latency.

- The two 1024-column half-batches are independent integrations; their stages
  are emitted interleaved (A-st0, B-st0, A-st1, ...) so each half's serial
  tail (layer3 -> k drain -> gating axpy -> next layer1) overlaps the other
  half's matmul work. All stage inputs X_j are finalized one full stage-slot
  before consumption.
- Stage inputs x_j = s + sum_m c_jm k_m are built by a late-formation chain
  into one scratch tile Q per half at stage j-2 (15 DVE axpys/step/half, no
  P partial tiles).
- cf = sum h B_m k_m accumulates inside the stage-6 layer-3 PSUM (W3 scaled
  by C*h*B6; k1/k3/k4/k5 via scaled-identity DoubleRow matmuls; B2=0), then
  one stt per PSUM tile updates the state: s += acc/C.
- The per-step +h*b3 state constant is pre-added as +8*h*b3 to the initial
  projection bias (the tiny transient x-perturbation is far below tolerance).
- k drains ride the Activation engine (Copy, scale=1/(WS*KS)); tanh drains
  emit fp8 with per-stage layer-1 biases folding b3's x-path correction.
"""

import numpy as np
import ml_dtypes

import concourse.bacc as bacc
import concourse.tile as tile
import concourse.mybir as mybir
from concourse.bass_utils import run_bass_kernel_spmd

F32 = mybir.dt.float32
F16 = mybir.dt.float16
F8 = mybir.dt.float8e4
AF = mybir.ActivationFunctionType
OP = mybir.AluOpType
DR = mybir.MatmulPerfMode.DoubleRow

N_CORES = 8
B, IN_DIM, HID = 16384, 256, 512
BSH = B // N_CORES
HALF = 1024
NSTEPS = 8
H = 0.1 * 1 / 8
WS = 16.0                   # fp8 weight scale
KS = 16.0                   # k tiles stored as m/KS

_A = (
    (1 / 5,),
    (3 / 40, 9 / 40),
    (44 / 45, -56 / 15, 32 / 9),
    (19372 / 6561, -25360 / 2187, 64448 / 6561, -212 / 729),
    (9017 / 3168, -355 / 33, 46732 / 5247, 49 / 176, -5103 / 18656),
)
_B = (35 / 384, 0.0, 500 / 1113, 125 / 192, -2187 / 6784, 11 / 84)

KB = HID // 128
KBP = IN_DIM // 128
NC = 512
CPH = HALF // NC

C_CF = 240.0 / (KS * H * _B[3])   # cf psum scale anchor


def build_nc(n_steps=NSTEPS):
    nc = bacc.Bacc("TRN2", target_bir_lowering=False, debug=False,
                   num_devices=N_CORES)

    yT = nc.declare_dram_parameter("yT", [HID, BSH], F32, isOutput=False)
    uT = nc.declare_dram_parameter("uT", [2 * IN_DIM, BSH], F16, isOutput=False)
    w1d = nc.declare_dram_parameter("w1", [16 * 128, 128], F8, isOutput=False)
    w2d = nc.declare_dram_parameter("w2", [16 * 128, 128], F8, isOutput=False)
    w3d = nc.declare_dram_parameter("w3", [16 * 128, 128], F8, isOutput=False)
    w3cd = nc.declare_dram_parameter("w3c", [16 * 128, 128], F8, isOutput=False)
    idd = nc.declare_dram_parameter("idw", [4 * 128, 128], F8, isOutput=False)
    wpd = nc.declare_dram_parameter("wp", [2 * IN_DIM, HID], F16, isOutput=False)
    bpd = nc.declare_dram_parameter("bp", [128, 4], F32, isOutput=False)
    b1ed = nc.declare_dram_parameter("b1e", [128, 24], F32, isOutput=False)
    b2d = nc.declare_dram_parameter("b2", [128, 4], F32, isOutput=False)
    outT = nc.declare_dram_parameter("outT", [HID, BSH], F32, isOutput=True)

    with tile.TileContext(nc) as tc:
        with (
            tc.tile_pool(name="wpool", bufs=1) as wp_,
            tc.tile_pool(name="spool", bufs=1) as sp,
            tc.tile_pool(name="pp", bufs=4, space="PSUM") as pp,
        ):
            # ---- resident weights/biases ----
            wpt = wp_.tile([128, 2 * KBP * 512], F16, tag="wp")
            for kb in range(2 * KBP):
                nc.gpsimd.dma_start(wpt[:, kb * 512:(kb + 1) * 512],
                                    wpd[kb * 128:(kb + 1) * 128, :])
            bpt = wp_.tile([128, 4], F32, tag="bp")
            b1et = wp_.tile([128, 24], F32, tag="b1e")
            b2t = wp_.tile([128, 4], F32, tag="b2")
            nc.gpsimd.dma_start(bpt[:], bpd[:])
            w1t = wp_.tile([128, 16, 128], F8, tag="w1")
            w2t = wp_.tile([128, 16, 128], F8, tag="w2")
            w3t = wp_.tile([128, 16, 128], F8, tag="w3")
            w3ct = wp_.tile([128, 16, 128], F8, tag="w3c")
            idp13 = wp_.tile([128, 2, 128], F8, tag="idp13")
            idp45 = wp_.tile([128, 2, 128], F8, tag="idp45")

            def load_weights():
                for j in range(16):
                    nc.gpsimd.dma_start(w1t[:, j, :],
                                        w1d[j * 128:(j + 1) * 128, :])
                nc.sync.dma_start(b1et[:], b1ed[:])
                for j in range(16):
                    nc.sync.dma_start(w2t[:, j, :],
                                      w2d[j * 128:(j + 1) * 128, :])
                nc.sync.dma_start(b2t[:], b2d[:])
                for j in range(16):
                    nc.gpsimd.dma_start(w3t[:, j, :],
                                        w3d[j * 128:(j + 1) * 128, :])
                for j in range(16):
                    nc.sync.dma_start(w3ct[:, j, :],
                                      w3cd[j * 128:(j + 1) * 128, :])
                nc.gpsimd.dma_start(idp13[:, 0, :], idd[0:128, :])
                nc.gpsimd.dma_start(idp13[:, 1, :], idd[128:256, :])
                nc.sync.dma_start(idp45[:, 0, :], idd[256:384, :])
                nc.sync.dma_start(idp45[:, 1, :], idd[384:512, :])

            # ---- per-half persistent state ----
            def half_state(hf):
                return dict(
                    s=sp.tile([128, KB, HALF], F32, tag=f"s{hf}",
                              name=f"s{hf}"),
                    s8=sp.tile([128, KB, HALF], F8, tag=f"s8{hf}",
                               name=f"s8{hf}"),
                    h1=sp.tile([128, KB, HALF], F8, tag=f"h1{hf}",
                               name=f"h1{hf}"),
                    h2=sp.tile([128, KB, HALF], F8, tag=f"h2{hf}",
                               name=f"h2{hf}"),
                    Qa=sp.tile([128, KB, HALF], F16, tag=f"Qa{hf}",
                               name=f"Qa{hf}"),
                    Qb=sp.tile([128, KB, HALF], F16, tag=f"Qb{hf}",
                               name=f"Qb{hf}"),
                    k8=sp.tile([128, 5 * KB, HALF], F8, tag=f"k8{hf}",
                               name=f"k8{hf}"),
                    X={j: sp.tile([128, KB, HALF], F8, tag=f"X{j}{hf}",
                                  name=f"X{j}{hf}")
                       for j in (2, 3, 4, 5, 6)},
                )

            HS = [half_state(0), half_state(1)]
            uTl = sp.tile([128, 2 * KBP, HALF], F16, tag="uTl")

            def kv(m):
                return (m - 1) * KB

            def layer(hs, groups, drain):
                for mb in range(4):
                    acc = pp.tile([128, HALF], F32, tag="psum", name="acc")
                    for c in range(CPH):
                        out = acc[:, c * NC:(c + 1) * NC]
                        ng = len(groups)
                        for gi, (wt, wb, xt, xb) in enumerate(groups):
                            for p in range(2):
                                wi = wb + p * 8 + mb * 2
                                nc.tensor.matmul(
                                    out,
                                    wt[:, wi:wi + 2, :],
                                    xt[:, xb + 2 * p:xb + 2 * p + 2,
                                       c * NC:(c + 1) * NC],
                                    start=(gi == 0 and p == 0),
                                    stop=(gi == ng - 1 and p == 1),
                                    perf_mode=DR)
                    drain(mb, acc)

            def cf_layer(hs, drain):
                h2, k8 = hs["h2"], hs["k8"]
                for mb in range(4):
                    acc = pp.tile([128, HALF], F32, tag="psum", name="acc")
                    for c in range(CPH):
                        out = acc[:, c * NC:(c + 1) * NC]
                        cs = slice(c * NC, (c + 1) * NC)
                        for p in range(2):
                            wi = p * 8 + mb * 2
                            nc.tensor.matmul(
                                out,
                                w3ct[:, wi:wi + 2, :],
                                h2[:, 2 * p:2 * p + 2, cs],
                                start=(p == 0), stop=False,
                                perf_mode=DR)
                        nc.tensor.matmul(
                            out, idp13[:],
                            k8[:, mb:2 * KB + mb + 1:2 * KB, cs],
                            start=False, stop=False, perf_mode=DR)
                        nc.tensor.matmul(
                            out, idp45[:],
                            k8[:, 3 * KB + mb:4 * KB + mb + 1:KB, cs],
                            start=False, stop=True, perf_mode=DR)
                    drain(mb, acc)

            def emit_proj(hs, c0):
                s, s8 = hs["s"], hs["s8"]
                pairs = [(0, 0), (1, 1), (2, 0), (3, 1)]
                for mb in range(4):
                    acc = pp.tile([128, HALF], F32, tag="psum", name="acc")
                    for c in range(CPH):
                        out = acc[:, c * NC:(c + 1) * NC]
                        for pi, (ub, wb) in enumerate(pairs):
                            nc.tensor.matmul(
                                out,
                                wpt[:, wb * 512 + mb * 128:
                                    wb * 512 + (mb + 1) * 128],
                                uTl[:, ub, c * NC:(c + 1) * NC],
                                start=(pi == 0), stop=(pi == len(pairs) - 1))
                    nc.vector.scalar_tensor_tensor(
                        s[:, mb, :], acc[:], bpt[:, mb:mb + 1], s[:, mb, :],
                        op0=OP.add, op1=OP.add)
                    nc.vector.tensor_copy(s8[:, mb, :], s[:, mb, :])

            def emit_l1(hs, st):
                s8, h1, X = hs["s8"], hs["h1"], hs["X"]
                g1 = [(w1t, 0, s8 if st == 0 else X[st + 1], 0)]

                def drain1(mb, acc, st=st):
                    nc.scalar.activation(
                        h1[:, mb, :], acc[:], AF.Tanh,
                        bias=b1et[:, st * 4 + mb:st * 4 + mb + 1],
                        scale=1.0 / WS)

                layer(hs, g1, drain1)

            def emit_l2(hs):
                h1, h2 = hs["h1"], hs["h2"]

                def drain2(mb, acc):
                    nc.scalar.activation(
                        h2[:, mb, :], acc[:], AF.Tanh,
                        bias=b2t[:, mb:mb + 1], scale=1.0 / WS)

                layer(hs, [(w2t, 0, h1, 0)], drain2)

            def emit_l3(hs, st, last, c0):
                s, s8, h2 = hs["s"], hs["s8"], hs["h2"]
                k8, X = hs["k8"], hs["X"]

                if st < 5:
                    def drain3(mb, acc, st=st):
                        nc.scalar.activation(
                            k8[:, kv(st + 1) + mb:kv(st + 1) + mb + 1, :],
                            acc[:], AF.Copy, bias=0.0, scale=1.0 / (WS * KS))
                    layer(hs, [(w3t, 0, h2, 0)], drain3)

                    # x_j chains spread flat across stages on two scratch
                    # tiles (same op sequence as the bunched form, so math is
                    # bit-identical); the consumer-critical X op goes first
                    Qa, Qb = hs["Qa"], hs["Qb"]

                    def km(m):
                        return k8[:, kv(m):kv(m) + KB, :]

                    def ax(dst, m, j, src_):
                        nc.vector.scalar_tensor_tensor(
                            dst[:], km(m), float(KS * H * _A[j - 2][m - 1]),
                            src_[:], op0=OP.mult, op1=OP.add)

                    if st == 0:
                        ax(X[2], 1, 2, s)
                        ax(Qa, 1, 3, s)
                    elif st == 1:
                        ax(X[3], 2, 3, Qa)
                        ax(Qb, 1, 4, s)
                        ax(Qb, 2, 4, Qb)
                    elif st == 2:
                        ax(X[4], 3, 4, Qb)
                        ax(Qa, 1, 5, s)
                        ax(Qa, 2, 5, Qa)
                        ax(Qa, 3, 5, Qa)
                    elif st == 3:
                        ax(X[5], 4, 5, Qa)
                        ax(Qb, 1, 6, s)
                        ax(Qb, 2, 6, Qb)
                        ax(Qb, 3, 6, Qb)
                    else:
                        ax(Qb, 4, 6, Qb)
                        ax(X[6], 5, 6, Qb)
                else:
                    def drain3(mb, acc, last=last, c0=c0):
                        nc.vector.scalar_tensor_tensor(
                            s[:, mb:mb + 1, :], acc[:], 1.0 / C_CF,
                            s[:, mb:mb + 1, :], op0=OP.mult, op1=OP.add)
                        if not last:
                            nc.vector.tensor_copy(
                                s8[:, mb:mb + 1, :], s[:, mb:mb + 1, :])
                        else:
                            nc.sync.dma_start(
                                outT[mb * 128:(mb + 1) * 128, c0:c0 + HALF],
                                s[:, mb:mb + 1, :])
                    cf_layer(hs, drain3)

            # ---- emission: proj for both halves, then interleaved stages ----
            for kb in range(2 * KBP):
                e = nc.gpsimd if kb % 2 == 0 else nc.sync
                e.dma_start(uTl[:, kb, :], uT[kb * 128:(kb + 1) * 128, 0:HALF])
            for kb in range(KB):
                e = nc.gpsimd if kb % 2 == 0 else nc.sync
                e.dma_start(HS[0]["s"][:, kb, :],
                            yT[kb * 128:(kb + 1) * 128, 0:HALF])
            load_weights()
            emit_proj(HS[0], 0)
            for kb in range(2 * KBP):
                e = nc.gpsimd if kb % 2 == 0 else nc.sync
                e.dma_start(uTl[:, kb, :],
                            uT[kb * 128:(kb + 1) * 128, HALF:2 * HALF])
            for kb in range(KB):
                e = nc.gpsimd if kb % 2 == 0 else nc.sync
                e.dma_start(HS[1]["s"][:, kb, :],
                            yT[kb * 128:(kb + 1) * 128, HALF:2 * HALF])
            emit_proj(HS[1], HALF)

            # stagger half B by 3 stage-slots so the two halves' expensive
            # step boundaries (state-update chains) alternate
            seq = [(step, st) for step in range(n_steps) for st in range(6)]
            OFF = 3
            for i in range(len(seq) + OFF):
                pair = [(0, i), (1, i - OFF)]
                valid = [(hf, ix) for hf, ix in pair if 0 <= ix < len(seq)]
                for hf, ix in valid:
                    emit_l1(HS[hf], seq[ix][1])
                for hf, ix in valid:
                    emit_l2(HS[hf])
                for hf, ix in valid:
                    step, st = seq[ix]
                    emit_l3(HS[hf], st, step == n_steps - 1, hf * HALF)

    nc.compile()
    return nc


_NC_CACHE = {}


def _get_nc(n_steps=NSTEPS):
    if n_steps not in _NC_CACHE:
        _NC_CACHE[n_steps] = build_nc(n_steps)
    return _NC_CACHE[n_steps]


def _make_in_maps(inputs):
    y = np.asarray(inputs["y"], np.float32)
    u_t = np.asarray(inputs["u_t"], np.float32)
    yT = np.ascontiguousarray(y.T)
    uT = np.ascontiguousarray(u_t.T)
    wp32 = np.asarray(inputs["Wp"], np.float32)
    wp_hi = wp32.astype(np.float16)
    wp_lo = (wp32 - wp_hi.astype(np.float32)).astype(np.float16)
    uT_hi = uT.astype(np.float16)
    uT_lo = (uT - uT_hi.astype(np.float32)).astype(np.float16)
    uT = np.concatenate([uT_hi, uT_lo], axis=0)

    def q8(w):
        return np.asarray(w, np.float64).astype(ml_dtypes.float8_e4m3)

    W1 = np.asarray(inputs["W1"], np.float64)
    W3 = np.asarray(inputs["W3"], np.float64)
    w1q = q8(WS * W1)
    w2q = q8(WS * np.asarray(inputs["W2"], np.float64))
    w3q = q8(WS * W3)
    w3c = q8(C_CF * H * _B[5] * W3)

    def dr_layout(wq):
        # [k, pair*8 + mb*2 + i, m] -> [2048, 128]; lhsT pair slices contiguous
        arr = np.zeros((16, 128, 128), wq.dtype)
        for p in range(2):
            for mb in range(4):
                for i in range(2):
                    arr[p * 8 + mb * 2 + i] = wq[(2 * p + i) * 128:
                                                 (2 * p + i + 1) * 128,
                                                 mb * 128:(mb + 1) * 128]
        return arr.reshape(16 * 128, 128)
    eye = np.eye(128, dtype=np.float64)
    idw = np.concatenate(
        [q8(C_CF * KS * H * _B[m - 1] * eye) for m in (1, 3, 4, 5)], axis=0)
    b1 = np.asarray(inputs["b1"], np.float64)
    b2 = np.asarray(inputs["b2"], np.float64)
    b3 = np.asarray(inputs["b3"], np.float64)
    b3w1 = b3 @ (w1q.astype(np.float64) / WS)
    b1e = np.zeros((6, HID), np.float64)
    b1e[0] = b1
    for st in range(1, 6):
        b1e[st] = b1 + H * float(sum(_A[st - 1])) * b3w1
    # per-step +h*b3 state constant pre-added to the projection bias
    bp_eff = (np.asarray(inputs["bp"], np.float64)
              + NSTEPS * H * b3).astype(np.float32)
    shared = {
        "w1": np.ascontiguousarray(dr_layout(w1q)),
        "w2": np.ascontiguousarray(dr_layout(w2q)),
        "w3": np.ascontiguousarray(dr_layout(w3q)),
        "w3c": np.ascontiguousarray(dr_layout(w3c)),
        "idw": np.ascontiguousarray(idw),
        "wp": np.ascontiguousarray(np.concatenate([wp_hi, wp_lo], axis=0)),
        "bp": np.ascontiguousarray(bp_eff.reshape(4, 128).T),
        "b1e": np.ascontiguousarray(
            b1e.reshape(6, 4, 128).transpose(2, 0, 1).reshape(128, 24)
            .astype(np.float32)),
        "b2": np.ascontiguousarray(b2.astype(np.float32).reshape(4, 128).T),
    }
    in_maps = []
    for i in range(N_CORES):
        sl = slice(i * BSH, (i + 1) * BSH)
        m = dict(shared)
        m["yT"] = np.ascontiguousarray(yT[:, sl])
        m["uT"] = np.ascontiguousarray(uT[:, sl])
        in_maps.append(m)
    return in_maps


def _run(inputs, trace=False, n_steps=NSTEPS):
    nc = _get_nc(n_steps)
    in_maps = _make_in_maps(inputs)
    res = run_bass_kernel_spmd(nc, in_maps, list(range(N_CORES)), trace=trace)
    out = np.empty((HID, B), np.float32)
    for i in range(N_CORES):
        out[:, i * BSH:(i + 1) * BSH] = res.results[i]["outT"]
    return np.ascontiguousarray(out.T), res


def kernel(**inputs) -> np.ndarray:
    out, _ = _run(inputs, trace=False)
    return out
